# revision 55
# baseline (speedup 1.0000x reference)
"""Multi-head self-attention (B=1, S=4096, D=512, H=8) on 8 trn2 NeuronCores.

Sharding: one head per core (head/tensor parallel). Each core computes its
head's Q/K/V projections from the full (transposed) query, runs attention
without materializing the full score matrix (streaming over key chunks,
softmax denominator via a ones-column augmented V^T), applies its slice of
out_proj fused with softmax normalization, and writes an unnormalized partial
[S, D] output. Host sums the 8 partials and adds out_proj bias.

v2: single fused software pipeline — qt DMA, K/V projections, attention,
out_proj and the output store all overlap. Q-projection for group g is
deferred into the previous attention window. Attention operands (Q, K, V, P)
are bf16 (1 cycle/row on the PE, fp8 was too lossy for the 2e-2 gate); wv is
loaded as bf16 so the narrow (N=64) V-proj matmuls avoid the 4x f32r
penalty. The scalar-engine exp stream is the critical path; the schedule
keeps it fed from the first microseconds.
"""

import sys

sys.path.insert(0, "/opt/trn_rl_repo")

import numpy as np

EMBED = 512
HEADS = 8
HD = 64          # head dim
S = 4096         # sequence length
P = 128          # partitions
NSK = S // P     # 32 key chunks of 128
QG = 512         # query group width (matmul free dim)
NQG = S // QG    # 8 query groups
NDC = EMBED // P # 4 contraction chunks for projections
SCALE = HD ** -0.5
EB = 3           # key chunks per exp batch (PSUM banks per s_ps buffer)
NB = (NSK + EB - 1) // EB  # 11 exp batches per query group

_compiled = {}

# test.py can set TRACE=True to capture an NTFF profile; LAST then holds the
# BassKernelResults. Off by default so the grading path is unchanged.
TRACE = False
LAST = None


def _build(n_cores=8, repeats=1):
    import concourse.bacc as bacc
    import concourse.mybir as mybir
    import concourse.tile as tile

    f32 = mybir.dt.float32
    f32r = mybir.dt.float32r
    bf16 = mybir.dt.bfloat16

    nc = bacc.Bacc("TRN2", target_bir_lowering=False, debug=False,
                   num_devices=n_cores)

    # query and in-proj weights ship as bf16 (host casts): halves the 8.4MB
    # query load — the phase-1 HBM floor — and satisfies walrus's
    # no-mixed-32/16-bit matmul rule for the narrow V-proj matmuls.
    qt = nc.dram_tensor("qt", [EMBED, S], bf16, kind="ExternalInput")
    # weights arrive pre-permuted to the SBUF layout so each loads in ONE
    # DMA instruction (HWDGE descriptor generation is a serialized ~630ns
    # per instruction and was gating phase 1)
    wq = nc.dram_tensor("wq", [P, NDC, HD], bf16, kind="ExternalInput")
    wk = nc.dram_tensor("wk", [P, NDC, HD], bf16, kind="ExternalInput")
    wv = nc.dram_tensor("wv", [P, NDC, HD], bf16, kind="ExternalInput")
    wo = nc.dram_tensor("wo", [HD, EMBED], f32r, kind="ExternalInput")
    bq = nc.dram_tensor("bq", [HD, 1], f32, kind="ExternalInput")
    bk = nc.dram_tensor("bk", [HD, 1], f32, kind="ExternalInput")
    bv = nc.dram_tensor("bv", [P, HD], f32, kind="ExternalInput")
    out_p = nc.dram_tensor("out_p", [S, EMBED], f32, kind="ExternalOutput")
    den = nc.dram_tensor("den", [1, S], f32, kind="ExternalOutput")

    with tile.TileContext(nc) as tc:
        for _ in range(repeats):
            _emit(tc, nc, mybir, qt, wq, wk, wv, wo, bq, bk, bv, out_p, den)

    nc.compile()
    return nc


def _emit(tc, nc, mybir, qt, wq, wk, wv, wo, bq, bk, bv, out_p, den):
    from contextlib import ExitStack

    f32 = mybir.dt.float32
    f32r = mybir.dt.float32r
    bf16 = mybir.dt.bfloat16
    Exp = mybir.ActivationFunctionType.Exp

    with ExitStack() as ctx:
        singles = ctx.enter_context(tc.tile_pool(name="singles", bufs=1))

        # warm up the ACT exp table while DMAs run
        warm = singles.tile([1, 1], f32)
        nc.vector.memset(warm, 0.0)
        warm2 = singles.tile([1, 1], f32)
        nc.scalar.activation(warm2, warm, Exp)

        # --- weights (wv cast to bf16 in the DMA: bf16 moving operand makes
        # the narrow V-proj matmuls run at 1 cycle/row instead of f32r's 4) ---
        wq_sb = singles.tile([P, NDC, HD], bf16)
        wk_sb = singles.tile([P, NDC, HD], bf16)
        wv_sb = singles.tile([P, NDC, HD], bf16)
        # everything on the sync queue: HWDGE generation is one shared serial
        # unit, and a DMA issued from nc.scalar would block the ACT queue
        nc.sync.dma_start(out=wk_sb, in_=wk[:, :, :])
        nc.sync.dma_start(out=wq_sb, in_=wq[:, :, :])
        nc.gpsimd.dma_start(out=wv_sb, in_=wv[:, :, :])
        bq_sb = singles.tile([HD, 1], f32)
        nc.gpsimd.dma_start(out=bq_sb, in_=bq[:, :])
        bk_sb = singles.tile([HD, 1], f32)
        nc.gpsimd.dma_start(out=bk_sb, in_=bk[:, :])
        bv_sb = singles.tile([P, HD], f32)
        nc.gpsimd.dma_start(out=bv_sb, in_=bv[:, :])

        wo_sb = singles.tile([HD, EMBED], f32r)

        # --- persistent activations (bf16) ---
        qh = singles.tile([HD, S], bf16)       # Q^T per head
        kh = singles.tile([HD, S], bf16)       # K^T per head
        vt_sb = singles.tile([P, NSK, HD + 1], bf16)  # V^T chunks + ones col
        ot_sb = singles.tile([HD, S], f32r)    # unnormalized attn out^T
        # softmax denominators, shipped to the host which applies the
        # normalization while summing the per-head partials — this keeps the
        # [1,S] -> [128, chunk] transpose DMAs and the normalize dependency
        # chain out of the device critical path entirely
        den_row = singles.tile([1, S], f32)

        nc.vector.memset(vt_sb[:, :, HD:HD + 1], 1.0)

        # full transposed query (bf16), loaded group-by-group for the cascade
        qt_sb = singles.tile([P, NDC, S], bf16)

        p_pool = ctx.enter_context(tc.tile_pool(name="p_pool", bufs=4))
        o_pool = ctx.enter_context(tc.tile_pool(name="o_pool", bufs=3))
        s_pool = ctx.enter_context(
            tc.tile_pool(name="s_pool", bufs=2, space="PSUM"))
        a_pool = ctx.enter_context(
            tc.tile_pool(name="a_pool", bufs=1, space="PSUM"))
        j_pool = ctx.enter_context(
            tc.tile_pool(name="j_pool", bufs=1, space="PSUM"))

        def dma_qt_group(g):
            gsl = slice(g * QG, (g + 1) * QG)
            for c in range(NDC):
                nc.sync.dma_start(
                    out=qt_sb[:, c, gsl],
                    in_=qt[c * P:(c + 1) * P, gsl])

        def proj_q(g, pool=None):
            gsl = slice(g * QG, (g + 1) * QG)
            pool = pool if pool is not None else j_pool
            tag = "pj" if pool is j_pool else "acc"
            accq = pool.tile([HD, QG], f32, tag=tag, name=f"accq{g}")
            for c in range(NDC):
                nc.tensor.matmul(accq, wq_sb[:, c, :], qt_sb[:, c, gsl],
                                 start=(c == 0), stop=(c == NDC - 1))
            nc.vector.tensor_scalar_add(qh[:, gsl], accq, bq_sb)

        def proj_qk0():
            # group 0 fast path: Q via the (still unused) a_pool slot so the
            # Q and K accumulations interleave per arriving qt chunk instead
            # of serializing through the single pj slot
            gsl = slice(0, QG)
            accq = a_pool.tile([HD, QG], f32, tag="acc", name="accq0")
            acck = j_pool.tile([HD, QG], f32, tag="pj", name="acck0")
            for c in range(NDC):
                nc.tensor.matmul(accq, wq_sb[:, c, :], qt_sb[:, c, gsl],
                                 start=(c == 0), stop=(c == NDC - 1))
                nc.tensor.matmul(acck, wk_sb[:, c, :], qt_sb[:, c, gsl],
                                 start=(c == 0), stop=(c == NDC - 1))
            nc.vector.tensor_scalar_add(qh[:, gsl], accq, bq_sb)
            nc.vector.tensor_scalar_add(kh[:, gsl], acck, bk_sb)

        def proj_k(g):
            gsl = slice(g * QG, (g + 1) * QG)
            acck = j_pool.tile([HD, QG], f32, tag="pj", name=f"acck{g}")
            for c in range(NDC):
                nc.tensor.matmul(acck, wk_sb[:, c, :], qt_sb[:, c, gsl],
                                 start=(c == 0), stop=(c == NDC - 1))
            nc.vector.tensor_scalar_add(kh[:, gsl], acck, bk_sb)

        def proj_v(g):
            # accv lives in an s_pool slot: keeps the pj slot free so the
            # next group's K-proj isn't serialized behind the V eviction
            accv = s_pool.tile([P, NDC, HD], f32, tag="sps", name=f"accv{g}")
            for i in range(NDC):
                for c in range(NDC):
                    nc.tensor.matmul(
                        accv[:, i, :],
                        qt_sb[:, c, g * QG + i * P:g * QG + (i + 1) * P],
                        wv_sb[:, c, :],
                        start=(c == 0), stop=(c == NDC - 1))
            for i in range(NDC):
                nc.vector.tensor_add(vt_sb[:, NDC * g + i, 0:HD], accv[:, i, :],
                                     bv_sb)

        # --- attention helpers ---
        st = {}

        def batch_chunks(b):
            return list(range(b * EB, min((b + 1) * EB, NSK)))

        def scores(qg, b):
            gsl = slice(qg * QG, (qg + 1) * QG)
            s_ps = s_pool.tile([P, EB, QG], f32, tag="sps", name=f"s{qg}_{b}")
            for i, s in enumerate(batch_chunks(b)):
                nc.tensor.matmul(
                    s_ps[:, i, :],
                    kh[:, s * P:(s + 1) * P], qh[:, gsl],
                    start=True, stop=True)
            st[(qg, b, "s")] = s_ps

        def expb(qg, b):
            nb = len(batch_chunks(b))
            pb = p_pool.tile([P, EB, QG], bf16, tag="p", name=f"p{qg}_{b}")
            nc.scalar.activation(pb[:, 0:nb, :], st[(qg, b, "s")][:, 0:nb, :],
                                 Exp, scale=SCALE)
            st[(qg, b, "p")] = pb

        def av(qg, b):
            chunks = batch_chunks(b)
            pb = st.pop((qg, b, "p"))
            st.pop((qg, b, "s"))
            out_acc = st[(qg, "acc")]
            for i, s in enumerate(chunks):
                nc.tensor.matmul(out_acc, vt_sb[:, s, :], pb[:, i, :],
                                 start=(s == 0), stop=(s == NSK - 1))

        def start_group(qg):
            st[(qg, "acc")] = a_pool.tile([HD + 1, QG], f32, tag="acc",
                                          name=f"oacc{qg}")

        def evict_group(qg):
            gsl = slice(qg * QG, (qg + 1) * QG)
            out_acc = st.pop((qg, "acc"))
            nc.vector.tensor_copy(den_row[:, gsl], out_acc[HD:HD + 1, :])
            nc.vector.tensor_copy(ot_sb[:, gsl], out_acc[0:HD, :])

        def stage_d_tile(t, pool=None):
            tsl = slice(t * P, (t + 1) * P)
            last = pool is not None
            pool = pool if pool is not None else j_pool
            tag = {id(j_pool): "pj", id(s_pool): "sps",
                   id(a_pool): "acc"}[id(pool)]
            o_ps = pool.tile([P, EMBED], f32, tag=tag, name=f"ops{t}")
            nc.tensor.matmul(o_ps, ot_sb[:, tsl], wo_sb,
                             start=True, stop=True)
            o_sb = o_pool.tile([P, EMBED], f32, tag="o", name=f"osb{t}")
            nc.vector.tensor_copy(o_sb, o_ps)
            # steady-state writes go via the idle Pool/SWDGE; the tail's
            # alternate across both queues to shorten the final drain
            (nc.sync if last and t % 2 == 0 else nc.gpsimd).dma_start(
                out=out_p[tsl, :], in_=o_sb)

        # ---- phase 1: qt DMA + projections, attention qg0 interleaved ----
        # batch b of qg0 is runnable after K/V of chunks <= its last chunk
        def pg(b):
            return (batch_chunks(b)[-1]) // NDC

        for g in range(NQG):
            dma_qt_group(g)
        nc.sync.dma_start(out=wo_sb, in_=wo[:, :])

        # dummy matmuls while the query streams in: brings the PE out of its
        # cold power state (and warms HAM on real hardware) so the first
        # projections run at full clock
        warm_sb = singles.tile([P, HD], f32r)
        nc.vector.memset(warm_sb.bitcast(f32), 0.0)
        warm_ps = j_pool.tile([HD, HD], f32, tag="pj", name="warm_ps")
        for _ in range(32):
            nc.tensor.matmul(warm_ps, warm_sb, warm_sb[:, 0:HD],
                             start=True, stop=True)

        proj_qk0()
        start_group(0)
        for g in range(NQG):
            if g > 0:
                proj_k(g)
            # scores before proj_v: the V accumulator shares s_pool slots,
            # and allocating it after the step's scores keeps the slot wait
            # off the exp-feeding path
            ready = [b for b in range(NB) if pg(b) == g]
            for b in ready:
                scores(0, b)
                expb(0, b)
            proj_v(g)
            for b in ready:
                if b >= 2:
                    av(0, b - 2)
        # Q-projections for the remaining groups: the PE idles here while
        # qg0's last exps drain, and phase 2's windows are PE-tight
        for g in range(1, NQG):
            proj_q(g)

        # ---- phase 2: attention qg 1..7, stage D folded one group behind.
        # Group boundaries overlap: the next group's first two scores are
        # emitted BEFORE the previous group's last two AVs, so the exp
        # stream never waits on the AV/evict chain.
        for qg in range(1, NQG):
            scores(qg, 0)
            expb(qg, 0)
            av(qg - 1, NB - 2)
            scores(qg, 1)
            expb(qg, 1)
            av(qg - 1, NB - 1)
            evict_group(qg - 1)
            start_group(qg)
            scores(qg, 2)
            expb(qg, 2)
            for b in range(3, NB + 1):
                j = b - 3
                if j % 2 == 0 and j // 2 < QG // P:
                    # out_proj + normalize + store for group qg-1
                    # (evicted at this window's boundary)
                    stage_d_tile((qg - 1) * (QG // P) + j // 2)
                if b <= NB - 1:
                    scores(qg, b)
                    expb(qg, b)
                av(qg, b - 3)
        av(NQG - 1, NB - 2)
        av(NQG - 1, NB - 1)
        evict_group(NQG - 1)
        nc.sync.dma_start(out=den[0:1, :], in_=den_row)
        # tail: spread the last group's four out_proj tiles across four
        # different PSUM slots so nothing waits on a rotation
        for i, pool in enumerate([s_pool, s_pool, j_pool, a_pool]):
            stage_d_tile((NQG - 1) * (QG // P) + i, pool=pool)


def _sb_layout(wt):
    """[D, HD] projection weight -> SBUF layout [P, NDC, HD], bf16."""
    import ml_dtypes

    return np.ascontiguousarray(
        wt.reshape(NDC, P, HD).transpose(1, 0, 2)).astype(ml_dtypes.bfloat16)


def _in_maps(query, in_proj_weight, in_proj_bias, out_proj_weight):
    import ml_dtypes

    q2d = np.asarray(query, dtype=np.float32).reshape(S, EMBED)
    qt = np.ascontiguousarray(q2d.T).astype(ml_dtypes.bfloat16)
    w = np.asarray(in_proj_weight, dtype=np.float32)
    b = np.asarray(in_proj_bias, dtype=np.float32)
    wout = np.asarray(out_proj_weight, dtype=np.float32)
    maps = []
    for h in range(HEADS):
        hs = slice(h * HD, (h + 1) * HD)
        maps.append({
            "qt": qt,
            "wq": _sb_layout(w[hs, :].T),
            "wk": _sb_layout(w[EMBED + h * HD:EMBED + (h + 1) * HD, :].T),
            "wv": _sb_layout(w[2 * EMBED + h * HD:2 * EMBED + (h + 1) * HD, :].T),
            "wo": np.ascontiguousarray(wout[:, hs].T),
            "bq": np.ascontiguousarray(b[hs].reshape(HD, 1)),
            "bk": np.ascontiguousarray(b[EMBED + h * HD:EMBED + (h + 1) * HD].reshape(HD, 1)),
            "bv": np.ascontiguousarray(
                np.broadcast_to(b[2 * EMBED + h * HD:2 * EMBED + (h + 1) * HD], (P, HD))),
        })
    return maps


def get_nc():
    if "nc" not in _compiled:
        _compiled["nc"] = _build()
    return _compiled["nc"]


def kernel(query, in_proj_weight, in_proj_bias, out_proj_weight, out_proj_bias):
    from concourse.bass_utils import run_bass_kernel_spmd

    nc = get_nc()
    maps = _in_maps(query, in_proj_weight, in_proj_bias, out_proj_weight)
    res = run_bass_kernel_spmd(nc, maps, core_ids=list(range(HEADS)),
                               trace=TRACE)
    global LAST
    LAST = res
    acc = np.zeros((S, EMBED), dtype=np.float32)
    for h in range(HEADS):
        # device partials are unnormalized; apply the per-head softmax
        # denominator here while summing
        acc += res.results[h]["out_p"] / res.results[h]["den"][0][:, None]
    acc += np.asarray(out_proj_bias, dtype=np.float32)[None, :]
    return acc.reshape(np.asarray(query).shape).astype(np.float32)


# revision 61
# speedup vs baseline: 1.0019x; 1.0019x over previous
"""Multi-head self-attention (B=1, S=4096, D=512, H=8) on 8 trn2 NeuronCores.

Sharding: one head per core (head/tensor parallel). Each core computes its
head's Q/K/V projections from the full (transposed) query, runs attention
without materializing the full score matrix (streaming over key chunks,
softmax denominator via a ones-column augmented V^T), applies its slice of
out_proj fused with softmax normalization, and writes an unnormalized partial
[S, D] output. Host sums the 8 partials and adds out_proj bias.

v2: single fused software pipeline — qt DMA, K/V projections, attention,
out_proj and the output store all overlap. Q-projection for group g is
deferred into the previous attention window. Attention operands (Q, K, V, P)
are bf16 (1 cycle/row on the PE, fp8 was too lossy for the 2e-2 gate); wv is
loaded as bf16 so the narrow (N=64) V-proj matmuls avoid the 4x f32r
penalty. The scalar-engine exp stream is the critical path; the schedule
keeps it fed from the first microseconds.
"""

import sys

sys.path.insert(0, "/opt/trn_rl_repo")

import numpy as np

EMBED = 512
HEADS = 8
HD = 64          # head dim
S = 4096         # sequence length
P = 128          # partitions
NSK = S // P     # 32 key chunks of 128
QG = 512         # query group width (matmul free dim)
NQG = S // QG    # 8 query groups
NDC = EMBED // P # 4 contraction chunks for projections
SCALE = HD ** -0.5
EB = 3           # key chunks per exp batch (PSUM banks per s_ps buffer)
NB = (NSK + EB - 1) // EB  # 11 exp batches per query group

_compiled = {}

# test.py can set TRACE=True to capture an NTFF profile; LAST then holds the
# BassKernelResults. Off by default so the grading path is unchanged.
TRACE = False
LAST = None


def _build(n_cores=8, repeats=1):
    import concourse.bacc as bacc
    import concourse.mybir as mybir
    import concourse.tile as tile

    f32 = mybir.dt.float32
    f32r = mybir.dt.float32r
    bf16 = mybir.dt.bfloat16

    nc = bacc.Bacc("TRN2", target_bir_lowering=False, debug=False,
                   num_devices=n_cores)

    # query and in-proj weights ship as bf16 (host casts): halves the 8.4MB
    # query load — the phase-1 HBM floor — and satisfies walrus's
    # no-mixed-32/16-bit matmul rule for the narrow V-proj matmuls.
    qt = nc.dram_tensor("qt", [EMBED, S], bf16, kind="ExternalInput")
    # weights arrive pre-permuted to the SBUF layout so each loads in ONE
    # DMA instruction (HWDGE descriptor generation is a serialized ~630ns
    # per instruction and was gating phase 1)
    wq = nc.dram_tensor("wq", [P, NDC, HD], bf16, kind="ExternalInput")
    wk = nc.dram_tensor("wk", [P, NDC, HD], bf16, kind="ExternalInput")
    wv = nc.dram_tensor("wv", [P, NDC, HD], bf16, kind="ExternalInput")
    wo = nc.dram_tensor("wo", [HD, EMBED], f32r, kind="ExternalInput")
    bq = nc.dram_tensor("bq", [HD, 1], f32, kind="ExternalInput")
    bk = nc.dram_tensor("bk", [HD, 1], f32, kind="ExternalInput")
    bv = nc.dram_tensor("bv", [P, HD], f32, kind="ExternalInput")
    out_p = nc.dram_tensor("out_p", [S, EMBED], f32, kind="ExternalOutput")
    den = nc.dram_tensor("den", [1, S], f32, kind="ExternalOutput")

    with tile.TileContext(nc) as tc:
        for _ in range(repeats):
            _emit(tc, nc, mybir, qt, wq, wk, wv, wo, bq, bk, bv, out_p, den)

    nc.compile()
    return nc


def _emit(tc, nc, mybir, qt, wq, wk, wv, wo, bq, bk, bv, out_p, den):
    from contextlib import ExitStack

    f32 = mybir.dt.float32
    f32r = mybir.dt.float32r
    bf16 = mybir.dt.bfloat16
    Exp = mybir.ActivationFunctionType.Exp

    with ExitStack() as ctx:
        singles = ctx.enter_context(tc.tile_pool(name="singles", bufs=1))

        # warm up the ACT exp table while DMAs run
        warm = singles.tile([1, 1], f32)
        nc.vector.memset(warm, 0.0)
        warm2 = singles.tile([1, 1], f32)
        nc.scalar.activation(warm2, warm, Exp)

        # --- weights (wv cast to bf16 in the DMA: bf16 moving operand makes
        # the narrow V-proj matmuls run at 1 cycle/row instead of f32r's 4) ---
        wq_sb = singles.tile([P, NDC, HD], bf16)
        wk_sb = singles.tile([P, NDC, HD], bf16)
        wv_sb = singles.tile([P, NDC, HD], bf16)
        # everything on the sync queue: HWDGE generation is one shared serial
        # unit, and a DMA issued from nc.scalar would block the ACT queue
        nc.sync.dma_start(out=wk_sb, in_=wk[:, :, :])
        nc.sync.dma_start(out=wq_sb, in_=wq[:, :, :])
        nc.gpsimd.dma_start(out=wv_sb, in_=wv[:, :, :])
        bq_sb = singles.tile([HD, 1], f32)
        nc.gpsimd.dma_start(out=bq_sb, in_=bq[:, :])
        bk_sb = singles.tile([HD, 1], f32)
        nc.gpsimd.dma_start(out=bk_sb, in_=bk[:, :])
        bv_sb = singles.tile([P, HD], f32)
        nc.gpsimd.dma_start(out=bv_sb, in_=bv[:, :])

        wo_sb = singles.tile([HD, EMBED], f32r)

        # --- persistent activations (bf16) ---
        qh = singles.tile([HD, S], bf16)       # Q^T per head
        kh = singles.tile([HD, S], bf16)       # K^T per head
        vt_sb = singles.tile([P, NSK, HD + 1], bf16)  # V^T chunks + ones col
        ot_sb = singles.tile([HD, S], f32r)    # unnormalized attn out^T
        # softmax denominators, shipped to the host which applies the
        # normalization while summing the per-head partials — this keeps the
        # [1,S] -> [128, chunk] transpose DMAs and the normalize dependency
        # chain out of the device critical path entirely
        den_row = singles.tile([1, S], f32)

        nc.vector.memset(vt_sb[:, :, HD:HD + 1], 1.0)

        # full transposed query (bf16), loaded group-by-group for the cascade
        qt_sb = singles.tile([P, NDC, S], bf16)

        p_pool = ctx.enter_context(tc.tile_pool(name="p_pool", bufs=4))
        o_pool = ctx.enter_context(tc.tile_pool(name="o_pool", bufs=3))
        s_pool = ctx.enter_context(
            tc.tile_pool(name="s_pool", bufs=2, space="PSUM"))
        a_pool = ctx.enter_context(
            tc.tile_pool(name="a_pool", bufs=1, space="PSUM"))
        j_pool = ctx.enter_context(
            tc.tile_pool(name="j_pool", bufs=1, space="PSUM"))

        def dma_qt_group(g):
            gsl = slice(g * QG, (g + 1) * QG)
            for c in range(NDC):
                nc.sync.dma_start(
                    out=qt_sb[:, c, gsl],
                    in_=qt[c * P:(c + 1) * P, gsl])

        def proj_q(g, pool=None):
            gsl = slice(g * QG, (g + 1) * QG)
            pool = pool if pool is not None else j_pool
            tag = "pj" if pool is j_pool else "acc"
            accq = pool.tile([HD, QG], f32, tag=tag, name=f"accq{g}")
            for c in range(NDC):
                nc.tensor.matmul(accq, wq_sb[:, c, :], qt_sb[:, c, gsl],
                                 start=(c == 0), stop=(c == NDC - 1))
            nc.vector.tensor_scalar_add(qh[:, gsl], accq, bq_sb)

        def proj_qk0():
            # group 0 fast path: Q via the (still unused) a_pool slot so the
            # Q and K accumulations interleave per arriving qt chunk instead
            # of serializing through the single pj slot
            gsl = slice(0, QG)
            accq = a_pool.tile([HD, QG], f32, tag="acc", name="accq0")
            acck = j_pool.tile([HD, QG], f32, tag="pj", name="acck0")
            for c in range(NDC):
                nc.tensor.matmul(accq, wq_sb[:, c, :], qt_sb[:, c, gsl],
                                 start=(c == 0), stop=(c == NDC - 1))
                nc.tensor.matmul(acck, wk_sb[:, c, :], qt_sb[:, c, gsl],
                                 start=(c == 0), stop=(c == NDC - 1))
            nc.vector.tensor_scalar_add(qh[:, gsl], accq, bq_sb)
            nc.vector.tensor_scalar_add(kh[:, gsl], acck, bk_sb)

        def proj_k(g):
            gsl = slice(g * QG, (g + 1) * QG)
            acck = j_pool.tile([HD, QG], f32, tag="pj", name=f"acck{g}")
            for c in range(NDC):
                nc.tensor.matmul(acck, wk_sb[:, c, :], qt_sb[:, c, gsl],
                                 start=(c == 0), stop=(c == NDC - 1))
            # evict per key-chunk so the first scores matmul of the next
            # batch starts after a 128-col copy instead of the full 512
            for i in range(NDC):
                nc.vector.tensor_scalar_add(
                    kh[:, g * QG + i * P:g * QG + (i + 1) * P],
                    acck[:, i * P:(i + 1) * P], bk_sb)

        def proj_v(g):
            # accv lives in an s_pool slot: keeps the pj slot free so the
            # next group's K-proj isn't serialized behind the V eviction
            accv = s_pool.tile([P, NDC, HD], f32, tag="sps", name=f"accv{g}")
            for i in range(NDC):
                for c in range(NDC):
                    nc.tensor.matmul(
                        accv[:, i, :],
                        qt_sb[:, c, g * QG + i * P:g * QG + (i + 1) * P],
                        wv_sb[:, c, :],
                        start=(c == 0), stop=(c == NDC - 1))
            for i in range(NDC):
                nc.vector.tensor_add(vt_sb[:, NDC * g + i, 0:HD], accv[:, i, :],
                                     bv_sb)

        # --- attention helpers ---
        st = {}

        def batch_chunks(b):
            return list(range(b * EB, min((b + 1) * EB, NSK)))

        def scores(qg, b):
            gsl = slice(qg * QG, (qg + 1) * QG)
            s_ps = s_pool.tile([P, EB, QG], f32, tag="sps", name=f"s{qg}_{b}")
            for i, s in enumerate(batch_chunks(b)):
                nc.tensor.matmul(
                    s_ps[:, i, :],
                    kh[:, s * P:(s + 1) * P], qh[:, gsl],
                    start=True, stop=True)
            st[(qg, b, "s")] = s_ps

        def expb(qg, b):
            nb = len(batch_chunks(b))
            pb = p_pool.tile([P, EB, QG], bf16, tag="p", name=f"p{qg}_{b}")
            nc.scalar.activation(pb[:, 0:nb, :], st[(qg, b, "s")][:, 0:nb, :],
                                 Exp, scale=SCALE)
            st[(qg, b, "p")] = pb

        def av(qg, b):
            chunks = batch_chunks(b)
            pb = st.pop((qg, b, "p"))
            st.pop((qg, b, "s"))
            out_acc = st[(qg, "acc")]
            for i, s in enumerate(chunks):
                nc.tensor.matmul(out_acc, vt_sb[:, s, :], pb[:, i, :],
                                 start=(s == 0), stop=(s == NSK - 1))

        def start_group(qg):
            st[(qg, "acc")] = a_pool.tile([HD + 1, QG], f32, tag="acc",
                                          name=f"oacc{qg}")

        def evict_group(qg):
            gsl = slice(qg * QG, (qg + 1) * QG)
            out_acc = st.pop((qg, "acc"))
            # evict ot per output row-tile so each out_proj matmul starts
            # after a quarter-width copy (shortens the final-group tail)
            for i in range(QG // P):
                nc.vector.tensor_copy(
                    ot_sb[:, qg * QG + i * P:qg * QG + (i + 1) * P],
                    out_acc[0:HD, i * P:(i + 1) * P])
            nc.vector.tensor_copy(den_row[:, gsl], out_acc[HD:HD + 1, :])

        def stage_d_tile(t, pool=None):
            tsl = slice(t * P, (t + 1) * P)
            last = pool is not None
            pool = pool if pool is not None else j_pool
            tag = {id(j_pool): "pj", id(s_pool): "sps",
                   id(a_pool): "acc"}[id(pool)]
            o_ps = pool.tile([P, EMBED], f32, tag=tag, name=f"ops{t}")
            nc.tensor.matmul(o_ps, ot_sb[:, tsl], wo_sb,
                             start=True, stop=True)
            o_sb = o_pool.tile([P, EMBED], f32, tag="o", name=f"osb{t}")
            nc.vector.tensor_copy(o_sb, o_ps)
            # steady-state writes go via the idle Pool/SWDGE; the tail's
            # alternate across both queues to shorten the final drain
            (nc.sync if last and t % 2 == 0 else nc.gpsimd).dma_start(
                out=out_p[tsl, :], in_=o_sb)

        # ---- phase 1: qt DMA + projections, attention qg0 interleaved ----
        # batch b of qg0 is runnable after K/V of chunks <= its last chunk
        def pg(b):
            return (batch_chunks(b)[-1]) // NDC

        for g in range(NQG):
            dma_qt_group(g)
        nc.sync.dma_start(out=wo_sb, in_=wo[:, :])

        # dummy matmuls while the query streams in: brings the PE out of its
        # cold power state (and warms HAM on real hardware) so the first
        # projections run at full clock
        warm_sb = singles.tile([P, HD], f32r)
        nc.vector.memset(warm_sb.bitcast(f32), 0.0)
        warm_ps = j_pool.tile([HD, HD], f32, tag="pj", name="warm_ps")
        for _ in range(32):
            nc.tensor.matmul(warm_ps, warm_sb, warm_sb[:, 0:HD],
                             start=True, stop=True)

        proj_qk0()
        start_group(0)
        for g in range(NQG):
            if g > 0:
                proj_k(g)
            # scores before proj_v: the V accumulator shares s_pool slots,
            # and allocating it after the step's scores keeps the slot wait
            # off the exp-feeding path
            ready = [b for b in range(NB) if pg(b) == g]
            for b in ready:
                scores(0, b)
                expb(0, b)
            proj_v(g)
            for b in ready:
                if b >= 2:
                    av(0, b - 2)
        # Q-projections for the remaining groups: the PE idles here while
        # qg0's last exps drain, and phase 2's windows are PE-tight
        for g in range(1, NQG):
            proj_q(g)

        # ---- phase 2: attention qg 1..7, stage D folded one group behind.
        # Group boundaries overlap: the next group's first two scores are
        # emitted BEFORE the previous group's last two AVs, so the exp
        # stream never waits on the AV/evict chain.
        for qg in range(1, NQG):
            scores(qg, 0)
            expb(qg, 0)
            av(qg - 1, NB - 2)
            scores(qg, 1)
            expb(qg, 1)
            av(qg - 1, NB - 1)
            evict_group(qg - 1)
            start_group(qg)
            scores(qg, 2)
            expb(qg, 2)
            for b in range(3, NB + 1):
                j = b - 3
                if j % 2 == 0 and j // 2 < QG // P:
                    # out_proj + normalize + store for group qg-1
                    # (evicted at this window's boundary)
                    stage_d_tile((qg - 1) * (QG // P) + j // 2)
                if b <= NB - 1:
                    scores(qg, b)
                    expb(qg, b)
                av(qg, b - 3)
        av(NQG - 1, NB - 2)
        av(NQG - 1, NB - 1)
        evict_group(NQG - 1)
        nc.sync.dma_start(out=den[0:1, :], in_=den_row)
        # tail: spread the last group's four out_proj tiles across four
        # different PSUM slots so nothing waits on a rotation
        for i, pool in enumerate([s_pool, s_pool, j_pool, a_pool]):
            stage_d_tile((NQG - 1) * (QG // P) + i, pool=pool)


def _sb_layout(wt):
    """[D, HD] projection weight -> SBUF layout [P, NDC, HD], bf16."""
    import ml_dtypes

    return np.ascontiguousarray(
        wt.reshape(NDC, P, HD).transpose(1, 0, 2)).astype(ml_dtypes.bfloat16)


def _in_maps(query, in_proj_weight, in_proj_bias, out_proj_weight):
    import ml_dtypes

    q2d = np.asarray(query, dtype=np.float32).reshape(S, EMBED)
    qt = np.ascontiguousarray(q2d.T).astype(ml_dtypes.bfloat16)
    w = np.asarray(in_proj_weight, dtype=np.float32)
    b = np.asarray(in_proj_bias, dtype=np.float32)
    wout = np.asarray(out_proj_weight, dtype=np.float32)
    maps = []
    for h in range(HEADS):
        hs = slice(h * HD, (h + 1) * HD)
        maps.append({
            "qt": qt,
            "wq": _sb_layout(w[hs, :].T),
            "wk": _sb_layout(w[EMBED + h * HD:EMBED + (h + 1) * HD, :].T),
            "wv": _sb_layout(w[2 * EMBED + h * HD:2 * EMBED + (h + 1) * HD, :].T),
            "wo": np.ascontiguousarray(wout[:, hs].T),
            "bq": np.ascontiguousarray(b[hs].reshape(HD, 1)),
            "bk": np.ascontiguousarray(b[EMBED + h * HD:EMBED + (h + 1) * HD].reshape(HD, 1)),
            "bv": np.ascontiguousarray(
                np.broadcast_to(b[2 * EMBED + h * HD:2 * EMBED + (h + 1) * HD], (P, HD))),
        })
    return maps


def get_nc():
    if "nc" not in _compiled:
        _compiled["nc"] = _build()
    return _compiled["nc"]


def kernel(query, in_proj_weight, in_proj_bias, out_proj_weight, out_proj_bias):
    from concourse.bass_utils import run_bass_kernel_spmd

    nc = get_nc()
    maps = _in_maps(query, in_proj_weight, in_proj_bias, out_proj_weight)
    res = run_bass_kernel_spmd(nc, maps, core_ids=list(range(HEADS)),
                               trace=TRACE)
    global LAST
    LAST = res
    acc = np.zeros((S, EMBED), dtype=np.float32)
    for h in range(HEADS):
        # device partials are unnormalized; apply the per-head softmax
        # denominator here while summing
        acc += res.results[h]["out_p"] / res.results[h]["den"][0][:, None]
    acc += np.asarray(out_proj_bias, dtype=np.float32)[None, :]
    return acc.reshape(np.asarray(query).shape).astype(np.float32)


# revision 73
# speedup vs baseline: 1.0027x; 1.0008x over previous
"""Multi-head self-attention (B=1, S=4096, D=512, H=8) on 8 trn2 NeuronCores.

Sharding: one head per core (head/tensor parallel). Each core computes its
head's Q/K/V projections from the full (transposed) query, runs attention
without materializing the full score matrix (streaming over key chunks,
softmax denominator via a ones-column augmented V^T), applies its slice of
out_proj fused with softmax normalization, and writes an unnormalized partial
[S, D] output. Host sums the 8 partials and adds out_proj bias.

v2: single fused software pipeline — qt DMA, K/V projections, attention,
out_proj and the output store all overlap. Q-projection for group g is
deferred into the previous attention window. Attention operands (Q, K, V, P)
are bf16 (1 cycle/row on the PE, fp8 was too lossy for the 2e-2 gate); wv is
loaded as bf16 so the narrow (N=64) V-proj matmuls avoid the 4x f32r
penalty. The scalar-engine exp stream is the critical path; the schedule
keeps it fed from the first microseconds.
"""

import sys

sys.path.insert(0, "/opt/trn_rl_repo")

import numpy as np

EMBED = 512
HEADS = 8
HD = 64          # head dim
S = 4096         # sequence length
P = 128          # partitions
NSK = S // P     # 32 key chunks of 128
QG = 512         # query group width (matmul free dim)
NQG = S // QG    # 8 query groups
NDC = EMBED // P # 4 contraction chunks for projections
SCALE = HD ** -0.5
EB = 3           # key chunks per exp batch (PSUM banks per s_ps buffer)
NB = (NSK + EB - 1) // EB  # 11 exp batches per query group

_compiled = {}

# test.py can set TRACE=True to capture an NTFF profile; LAST then holds the
# BassKernelResults. Off by default so the grading path is unchanged.
TRACE = False
LAST = None


def _build(n_cores=8, repeats=1):
    import concourse.bacc as bacc
    import concourse.mybir as mybir
    import concourse.tile as tile

    f32 = mybir.dt.float32
    f32r = mybir.dt.float32r
    bf16 = mybir.dt.bfloat16

    nc = bacc.Bacc("TRN2", target_bir_lowering=False, debug=False,
                   num_devices=n_cores)

    # query and in-proj weights ship as bf16 (host casts): halves the 8.4MB
    # query load — the phase-1 HBM floor — and satisfies walrus's
    # no-mixed-32/16-bit matmul rule for the narrow V-proj matmuls.
    qt = nc.dram_tensor("qt", [EMBED, S], bf16, kind="ExternalInput")
    # weights arrive pre-permuted to the SBUF layout so each loads in ONE
    # DMA instruction (HWDGE descriptor generation is a serialized ~630ns
    # per instruction and was gating phase 1)
    wq = nc.dram_tensor("wq", [P, NDC, HD], bf16, kind="ExternalInput")
    wk = nc.dram_tensor("wk", [P, NDC, HD], bf16, kind="ExternalInput")
    wv = nc.dram_tensor("wv", [P, NDC, HD], bf16, kind="ExternalInput")
    wo = nc.dram_tensor("wo", [HD, EMBED], f32r, kind="ExternalInput")
    bq = nc.dram_tensor("bq", [HD, 1], f32, kind="ExternalInput")
    bk = nc.dram_tensor("bk", [HD, 1], f32, kind="ExternalInput")
    bv = nc.dram_tensor("bv", [P, HD], f32, kind="ExternalInput")
    out_p = nc.dram_tensor("out_p", [S, EMBED], f32, kind="ExternalOutput")
    den = nc.dram_tensor("den", [1, S], f32, kind="ExternalOutput")

    with tile.TileContext(nc) as tc:
        for _ in range(repeats):
            _emit(tc, nc, mybir, qt, wq, wk, wv, wo, bq, bk, bv, out_p, den)

    nc.compile()
    return nc


def _emit(tc, nc, mybir, qt, wq, wk, wv, wo, bq, bk, bv, out_p, den):
    from contextlib import ExitStack

    f32 = mybir.dt.float32
    f32r = mybir.dt.float32r
    bf16 = mybir.dt.bfloat16
    Exp = mybir.ActivationFunctionType.Exp

    with ExitStack() as ctx:
        singles = ctx.enter_context(tc.tile_pool(name="singles", bufs=1))

        # warm up the ACT exp table while DMAs run
        warm = singles.tile([1, 1], f32)
        nc.vector.memset(warm, 0.0)
        warm2 = singles.tile([1, 1], f32)
        nc.scalar.activation(warm2, warm, Exp)

        # --- weights (wv cast to bf16 in the DMA: bf16 moving operand makes
        # the narrow V-proj matmuls run at 1 cycle/row instead of f32r's 4) ---
        wq_sb = singles.tile([P, NDC, HD], bf16)
        wk_sb = singles.tile([P, NDC, HD], bf16)
        wv_sb = singles.tile([P, NDC, HD], bf16)
        # everything on the sync queue: HWDGE generation is one shared serial
        # unit, and a DMA issued from nc.scalar would block the ACT queue
        nc.sync.dma_start(out=wk_sb, in_=wk[:, :, :])
        nc.sync.dma_start(out=wq_sb, in_=wq[:, :, :])
        nc.gpsimd.dma_start(out=wv_sb, in_=wv[:, :, :])
        bq_sb = singles.tile([HD, 1], f32)
        nc.gpsimd.dma_start(out=bq_sb, in_=bq[:, :])
        bk_sb = singles.tile([HD, 1], f32)
        nc.gpsimd.dma_start(out=bk_sb, in_=bk[:, :])
        bv_sb = singles.tile([P, HD], f32)
        nc.gpsimd.dma_start(out=bv_sb, in_=bv[:, :])

        wo_sb = singles.tile([HD, EMBED], f32r)

        # --- persistent activations (bf16) ---
        qh = singles.tile([HD, S], bf16)       # Q^T per head
        kh = singles.tile([HD, S], bf16)       # K^T per head
        vt_sb = singles.tile([P, NSK, HD + 1], bf16)  # V^T chunks + ones col
        ot_sb = singles.tile([HD, S], f32r)    # unnormalized attn out^T
        # softmax denominators, shipped to the host which applies the
        # normalization while summing the per-head partials — this keeps the
        # [1,S] -> [128, chunk] transpose DMAs and the normalize dependency
        # chain out of the device critical path entirely
        den_row = singles.tile([1, S], f32)

        nc.vector.memset(vt_sb[:, :, HD:HD + 1], 1.0)

        # full transposed query (bf16), loaded group-by-group for the cascade
        qt_sb = singles.tile([P, NDC, S], bf16)

        p_pool = ctx.enter_context(tc.tile_pool(name="p_pool", bufs=4))
        o_pool = ctx.enter_context(tc.tile_pool(name="o_pool", bufs=3))
        s_pool = ctx.enter_context(
            tc.tile_pool(name="s_pool", bufs=2, space="PSUM"))
        a_pool = ctx.enter_context(
            tc.tile_pool(name="a_pool", bufs=1, space="PSUM"))
        j_pool = ctx.enter_context(
            tc.tile_pool(name="j_pool", bufs=1, space="PSUM"))

        def dma_qt_group(g):
            gsl = slice(g * QG, (g + 1) * QG)
            for c in range(NDC):
                nc.sync.dma_start(
                    out=qt_sb[:, c, gsl],
                    in_=qt[c * P:(c + 1) * P, gsl])

        def proj_q(g, pool=None):
            gsl = slice(g * QG, (g + 1) * QG)
            pool = pool if pool is not None else j_pool
            tag = "pj" if pool is j_pool else "acc"
            accq = pool.tile([HD, QG], f32, tag=tag, name=f"accq{g}")
            for c in range(NDC):
                nc.tensor.matmul(accq, wq_sb[:, c, :], qt_sb[:, c, gsl],
                                 start=(c == 0), stop=(c == NDC - 1))
            nc.vector.tensor_scalar_add(qh[:, gsl], accq, bq_sb)

        def proj_qk0():
            # group 0 fast path: Q via the (still unused) a_pool slot so the
            # Q and K accumulations interleave per arriving qt chunk instead
            # of serializing through the single pj slot
            gsl = slice(0, QG)
            accq = a_pool.tile([HD, QG], f32, tag="acc", name="accq0")
            acck = j_pool.tile([HD, QG], f32, tag="pj", name="acck0")
            for c in range(NDC):
                nc.tensor.matmul(accq, wq_sb[:, c, :], qt_sb[:, c, gsl],
                                 start=(c == 0), stop=(c == NDC - 1))
                nc.tensor.matmul(acck, wk_sb[:, c, :], qt_sb[:, c, gsl],
                                 start=(c == 0), stop=(c == NDC - 1))
            nc.vector.tensor_scalar_add(qh[:, gsl], accq, bq_sb)
            # K evicted per chunk: scores(0,0)'s first matmul needs only
            # chunk 0, shaving the very first exp's latency chain
            for i in range(NDC):
                nc.vector.tensor_scalar_add(
                    kh[:, i * P:(i + 1) * P], acck[:, i * P:(i + 1) * P],
                    bk_sb)

        def proj_k(g):
            gsl = slice(g * QG, (g + 1) * QG)
            acck = j_pool.tile([HD, QG], f32, tag="pj", name=f"acck{g}")
            for c in range(NDC):
                nc.tensor.matmul(acck, wk_sb[:, c, :], qt_sb[:, c, gsl],
                                 start=(c == 0), stop=(c == NDC - 1))
            # evict per key-chunk so the first scores matmul of the next
            # batch starts after a 128-col copy instead of the full 512
            for i in range(NDC):
                nc.vector.tensor_scalar_add(
                    kh[:, g * QG + i * P:g * QG + (i + 1) * P],
                    acck[:, i * P:(i + 1) * P], bk_sb)

        def proj_v(g):
            # accv lives in an s_pool slot: keeps the pj slot free so the
            # next group's K-proj isn't serialized behind the V eviction
            accv = s_pool.tile([P, NDC, HD], f32, tag="sps", name=f"accv{g}")
            for i in range(NDC):
                for c in range(NDC):
                    nc.tensor.matmul(
                        accv[:, i, :],
                        qt_sb[:, c, g * QG + i * P:g * QG + (i + 1) * P],
                        wv_sb[:, c, :],
                        start=(c == 0), stop=(c == NDC - 1))
            for i in range(NDC):
                nc.vector.tensor_add(vt_sb[:, NDC * g + i, 0:HD], accv[:, i, :],
                                     bv_sb)

        # --- attention helpers ---
        st = {}

        def batch_chunks(b):
            return list(range(b * EB, min((b + 1) * EB, NSK)))

        def scores(qg, b):
            gsl = slice(qg * QG, (qg + 1) * QG)
            s_ps = s_pool.tile([P, EB, QG], f32, tag="sps", name=f"s{qg}_{b}")
            for i, s in enumerate(batch_chunks(b)):
                nc.tensor.matmul(
                    s_ps[:, i, :],
                    kh[:, s * P:(s + 1) * P], qh[:, gsl],
                    start=True, stop=True)
            st[(qg, b, "s")] = s_ps

        def expb(qg, b):
            nb = len(batch_chunks(b))
            pb = p_pool.tile([P, EB, QG], bf16, tag="p", name=f"p{qg}_{b}")
            nc.scalar.activation(pb[:, 0:nb, :], st[(qg, b, "s")][:, 0:nb, :],
                                 Exp, scale=SCALE)
            st[(qg, b, "p")] = pb

        def av(qg, b):
            chunks = batch_chunks(b)
            pb = st.pop((qg, b, "p"))
            st.pop((qg, b, "s"))
            out_acc = st[(qg, "acc")]
            for i, s in enumerate(chunks):
                nc.tensor.matmul(out_acc, vt_sb[:, s, :], pb[:, i, :],
                                 start=(s == 0), stop=(s == NSK - 1))

        def start_group(qg):
            st[(qg, "acc")] = a_pool.tile([HD + 1, QG], f32, tag="acc",
                                          name=f"oacc{qg}")

        def evict_group(qg):
            gsl = slice(qg * QG, (qg + 1) * QG)
            out_acc = st.pop((qg, "acc"))
            # evict ot per output row-tile so each out_proj matmul starts
            # after a quarter-width copy (shortens the final-group tail)
            for i in range(QG // P):
                nc.vector.tensor_copy(
                    ot_sb[:, qg * QG + i * P:qg * QG + (i + 1) * P],
                    out_acc[0:HD, i * P:(i + 1) * P])
            nc.vector.tensor_copy(den_row[:, gsl], out_acc[HD:HD + 1, :])

        def stage_d_tile(t, pool=None):
            tsl = slice(t * P, (t + 1) * P)
            last = pool is not None
            pool = pool if pool is not None else j_pool
            tag = {id(j_pool): "pj", id(s_pool): "sps",
                   id(a_pool): "acc"}[id(pool)]
            o_ps = pool.tile([P, EMBED], f32, tag=tag, name=f"ops{t}")
            nc.tensor.matmul(o_ps, ot_sb[:, tsl], wo_sb,
                             start=True, stop=True)
            o_sb = o_pool.tile([P, EMBED], f32, tag="o", name=f"osb{t}")
            nc.vector.tensor_copy(o_sb, o_ps)
            # steady-state writes go via the idle Pool/SWDGE; the tail's
            # alternate across both queues to shorten the final drain
            (nc.sync if last and t % 2 == 0 else nc.gpsimd).dma_start(
                out=out_p[tsl, :], in_=o_sb)

        # ---- phase 1: qt DMA + projections, attention qg0 interleaved ----
        # batch b of qg0 is runnable after K/V of chunks <= its last chunk
        def pg(b):
            return (batch_chunks(b)[-1]) // NDC

        for g in range(NQG):
            dma_qt_group(g)
        nc.sync.dma_start(out=wo_sb, in_=wo[:, :])

        # dummy matmuls while the query streams in: brings the PE out of its
        # cold power state (and warms HAM on real hardware) so the first
        # projections run at full clock
        warm_sb = singles.tile([P, HD], f32r)
        nc.vector.memset(warm_sb.bitcast(f32), 0.0)
        warm_ps = j_pool.tile([HD, HD], f32, tag="pj", name="warm_ps")
        for _ in range(32):
            nc.tensor.matmul(warm_ps, warm_sb, warm_sb[:, 0:HD],
                             start=True, stop=True)

        proj_qk0()
        start_group(0)
        for g in range(NQG):
            if g > 0:
                proj_k(g)
            # scores before proj_v: the V accumulator shares s_pool slots,
            # and allocating it after the step's scores keeps the slot wait
            # off the exp-feeding path
            ready = [b for b in range(NB) if pg(b) == g]
            for b in ready:
                scores(0, b)
                expb(0, b)
            proj_v(g)
            for b in ready:
                if b >= 2:
                    av(0, b - 2)
        # Q-projections for the remaining groups: the PE idles here while
        # qg0's last exps drain, and phase 2's windows are PE-tight
        for g in range(1, NQG):
            proj_q(g)

        # ---- phase 2: attention qg 1..7, stage D folded one group behind.
        # Group boundaries overlap: the next group's first two scores are
        # emitted BEFORE the previous group's last two AVs, so the exp
        # stream never waits on the AV/evict chain.
        for qg in range(1, NQG):
            scores(qg, 0)
            expb(qg, 0)
            av(qg - 1, NB - 2)
            scores(qg, 1)
            expb(qg, 1)
            av(qg - 1, NB - 1)
            evict_group(qg - 1)
            start_group(qg)
            scores(qg, 2)
            expb(qg, 2)
            for b in range(3, NB + 1):
                j = b - 3
                if j % 2 == 0 and j // 2 < QG // P:
                    # out_proj + normalize + store for group qg-1
                    # (evicted at this window's boundary)
                    stage_d_tile((qg - 1) * (QG // P) + j // 2)
                if b <= NB - 1:
                    scores(qg, b)
                    expb(qg, b)
                av(qg, b - 3)
        av(NQG - 1, NB - 2)
        av(NQG - 1, NB - 1)
        evict_group(NQG - 1)
        nc.sync.dma_start(out=den[0:1, :], in_=den_row)
        # tail: spread the last group's four out_proj tiles across four
        # different PSUM slots so nothing waits on a rotation
        for i, pool in enumerate([s_pool, s_pool, j_pool, a_pool]):
            stage_d_tile((NQG - 1) * (QG // P) + i, pool=pool)


def _sb_layout(wt):
    """[D, HD] projection weight -> SBUF layout [P, NDC, HD], bf16."""
    import ml_dtypes

    return np.ascontiguousarray(
        wt.reshape(NDC, P, HD).transpose(1, 0, 2)).astype(ml_dtypes.bfloat16)


def _in_maps(query, in_proj_weight, in_proj_bias, out_proj_weight):
    import ml_dtypes

    q2d = np.asarray(query, dtype=np.float32).reshape(S, EMBED)
    qt = np.ascontiguousarray(q2d.T).astype(ml_dtypes.bfloat16)
    w = np.asarray(in_proj_weight, dtype=np.float32)
    b = np.asarray(in_proj_bias, dtype=np.float32)
    wout = np.asarray(out_proj_weight, dtype=np.float32)
    maps = []
    for h in range(HEADS):
        hs = slice(h * HD, (h + 1) * HD)
        maps.append({
            "qt": qt,
            "wq": _sb_layout(w[hs, :].T),
            "wk": _sb_layout(w[EMBED + h * HD:EMBED + (h + 1) * HD, :].T),
            "wv": _sb_layout(w[2 * EMBED + h * HD:2 * EMBED + (h + 1) * HD, :].T),
            "wo": np.ascontiguousarray(wout[:, hs].T),
            "bq": np.ascontiguousarray(b[hs].reshape(HD, 1)),
            "bk": np.ascontiguousarray(b[EMBED + h * HD:EMBED + (h + 1) * HD].reshape(HD, 1)),
            "bv": np.ascontiguousarray(
                np.broadcast_to(b[2 * EMBED + h * HD:2 * EMBED + (h + 1) * HD], (P, HD))),
        })
    return maps


def get_nc():
    if "nc" not in _compiled:
        _compiled["nc"] = _build()
    return _compiled["nc"]


def kernel(query, in_proj_weight, in_proj_bias, out_proj_weight, out_proj_bias):
    from concourse.bass_utils import run_bass_kernel_spmd

    nc = get_nc()
    maps = _in_maps(query, in_proj_weight, in_proj_bias, out_proj_weight)
    res = run_bass_kernel_spmd(nc, maps, core_ids=list(range(HEADS)),
                               trace=TRACE)
    global LAST
    LAST = res
    acc = np.zeros((S, EMBED), dtype=np.float32)
    for h in range(HEADS):
        # device partials are unnormalized; apply the per-head softmax
        # denominator here while summing
        acc += res.results[h]["out_p"] / res.results[h]["den"][0][:, None]
    acc += np.asarray(out_proj_bias, dtype=np.float32)[None, :]
    return acc.reshape(np.asarray(query).shape).astype(np.float32)
